# revision 1
# baseline (speedup 1.0000x reference)
"""Trainium2 Bass kernel for nn_LrFeatureUpScaler (2-layer TransformerConv GNN).

Sharding over 8 NeuronCores:
  conv1 (4 heads): core i = (head i//2, node-half i%2). Each core computes its
    head's k/v over all nodes, q/skip for its 512 target nodes, in transposed
    [feature, node] layout. One AllGather shares pre-norm h1 blocks + GraphNorm1
    partial sums; each core then normalizes full h1 locally.
  conv2 (8 heads): core i = head i; fully local. GraphNorm2 local.
  Final row-normalize: tiny AllGather of per-node partial sum-squares.
"""
import numpy as np
import ml_dtypes

import concourse.bass as bass
import concourse.mybir as mybir
import concourse.tile as tile
from concourse.bass_utils import run_bass_kernel_spmd
from concourse.masks import make_identity

N = 1024
HR = 2048
EPS = 1e-5
INV_S = float(1.0 / np.sqrt(512.0))
F32 = mybir.dt.float32
BF16 = mybir.dt.bfloat16
F32R = mybir.dt.float32r
AF = mybir.ActivationFunctionType
ALU = mybir.AluOpType
AX = mybir.AxisListType
N_CORES = 8


def mmf(nc, ps, l, r, start, stop):
    nc.tensor.matmul(ps, l, r, start=start, stop=stop)


def mmb(nc, ps, l, r, start, stop):
    nc.tensor.matmul(ps, l, r, start=start, stop=stop)


def build_nc():
    nc = bass.Bass()
    # ---- I/O ----
    xT = nc.dram_tensor("xT", [N, N], BF16, kind="ExternalInput")
    xt_tgt = nc.dram_tensor("xt_tgt", [N, 512], BF16, kind="ExternalInput")
    x_edge = nc.dram_tensor("x_edge", [512, N], BF16, kind="ExternalInput")
    wq1 = nc.dram_tensor("wq1", [N, 512], BF16, kind="ExternalInput")
    wk1 = nc.dram_tensor("wk1", [N, 512], BF16, kind="ExternalInput")
    wv1 = nc.dram_tensor("wv1", [N, 512], BF16, kind="ExternalInput")
    ws1 = nc.dram_tensor("ws1", [N, 512], BF16, kind="ExternalInput")
    bq1 = nc.dram_tensor("bq1", [512], F32, kind="ExternalInput")
    bk1 = nc.dram_tensor("bk1", [512], F32, kind="ExternalInput")
    bvs1 = nc.dram_tensor("bvs1", [512], F32, kind="ExternalInput")
    we1 = nc.dram_tensor("we1", [512], F32, kind="ExternalInput")
    gn1g = nc.dram_tensor("gn1g", [HR], F32, kind="ExternalInput")
    gn1b = nc.dram_tensor("gn1b", [HR], F32, kind="ExternalInput")
    gn1m = nc.dram_tensor("gn1m", [HR], F32, kind="ExternalInput")
    wq2 = nc.dram_tensor("wq2", [HR, 512], BF16, kind="ExternalInput")
    wk2 = nc.dram_tensor("wk2", [HR, 512], BF16, kind="ExternalInput")
    wv2 = nc.dram_tensor("wv2", [HR, 512], BF16, kind="ExternalInput")
    ws2 = nc.dram_tensor("ws2", [HR, 512], BF16, kind="ExternalInput")
    bq2 = nc.dram_tensor("bq2", [512], F32, kind="ExternalInput")
    bk2 = nc.dram_tensor("bk2", [512], F32, kind="ExternalInput")
    bvs2 = nc.dram_tensor("bvs2", [512], F32, kind="ExternalInput")
    we2 = nc.dram_tensor("we2", [512], F32, kind="ExternalInput")
    gn2g = nc.dram_tensor("gn2g", [512], F32, kind="ExternalInput")
    gn2b = nc.dram_tensor("gn2b", [512], F32, kind="ExternalInput")
    gn2m = nc.dram_tensor("gn2m", [512], F32, kind="ExternalInput")
    out = nc.dram_tensor("out", [N, 512], F32, kind="ExternalOutput")

    with tile.TileContext(nc) as tc:
        with (
            tc.tile_pool(name="const", bufs=1) as cp,
            tc.tile_pool(name="xp", bufs=1) as xp,
            tc.tile_pool(name="h1p", bufs=1) as h1p,
            tc.tile_pool(name="big", bufs=1) as bigp,
            tc.tile_pool(name="wc", bufs=24) as wc,
            tc.tile_pool(name="sm", bufs=2) as sm,
            tc.tile_pool(name="smc", bufs=1) as smc,
            tc.tile_pool(name="dram", bufs=1, space="DRAM") as dp,
            tc.tile_pool(name="pp", bufs=4, space="PSUM") as pp,
            tc.tile_pool(name="ppt", bufs=2, space="PSUM") as ppt,
            tc.tile_pool(name="pps", bufs=2, space="PSUM") as pps,
        ):
            # ---------- constants ----------
            ident = cp.tile([128, 128], F32, name="ident")
            make_identity(nc, ident[:, :])
            ones_col = cp.tile([128, 1], BF16, name="ones_col")
            nc.gpsimd.memset(ones_col[:, :], 1.0)
            eps_col = cp.tile([128, 1], F32, name="eps_col")
            nc.gpsimd.memset(eps_col[:, :], EPS)

            def vec_cols(t, w, nm):
                s = cp.tile([128, w], F32, name=nm)
                nc.sync.dma_start(s[:, :], t.rearrange("(a p) -> p a", p=128))
                return s

            bq1c = vec_cols(bq1, 4, "bq1c")
            bk1c = vec_cols(bk1, 4, "bk1c")
            bvs1c = vec_cols(bvs1, 4, "bvs1c")
            we1c = vec_cols(we1, 4, "we1c")
            bq2c = vec_cols(bq2, 4, "bq2c")
            bk2c = vec_cols(bk2, 4, "bk2c")
            bvs2c = vec_cols(bvs2, 4, "bvs2c")
            we2c = vec_cols(we2, 4, "we2c")
            gn1gc = vec_cols(gn1g, 16, "gn1gc")
            gn1bc = vec_cols(gn1b, 16, "gn1bc")
            gn1mc = vec_cols(gn1m, 16, "gn1mc")
            gn2gc = vec_cols(gn2g, 4, "gn2gc")
            gn2bc = vec_cols(gn2b, 4, "gn2bc")
            gn2mc = vec_cols(gn2m, 4, "gn2mc")
            we1cb = cp.tile([128, 4], BF16, name="we1cb")
            nc.vector.tensor_copy(we1cb[:, :], we1c[:, :])
            we2cb = cp.tile([128, 4], BF16, name="we2cb")
            nc.vector.tensor_copy(we2cb[:, :], we2c[:, :])
            we1r = cp.tile([1, 512], F32, name="we1r")
            nc.sync.dma_start(we1r[0:1, :], we1.rearrange("(o f) -> o f", o=1))
            we2r = cp.tile([1, 512], F32, name="we2r")
            nc.sync.dma_start(we2r[0:1, :], we2.rearrange("(o f) -> o f", o=1))
            we1rb = cp.tile([1, 512], BF16, name="we1rb")
            nc.vector.tensor_copy(we1rb[0:1, :], we1r[0:1, :])
            we2rb = cp.tile([1, 512], BF16, name="we2rb")
            nc.vector.tensor_copy(we2rb[0:1, :], we2r[0:1, :])

            # ---------- x loads ----------
            xT_sb = []
            for fc in range(8):
                t = xp.tile([128, N], BF16, name=f"xT{fc}")
                nc.sync.dma_start(t[:, :], xT[fc * 128:(fc + 1) * 128, :])
                xT_sb.append(t)
            h1T = [h1p.tile([128, N], BF16, name=f"h1T{f}") for f in range(16)]

            # DRAM collective buffers
            ag_in = dp.tile([514, 512], F32, name="ag_in")
            ag_out = dp.tile([8, 514, 512], F32, name="ag_out", addr_space="Shared")
            rn_in = dp.tile([1, N], F32, name="rn_in")
            rn_out = dp.tile([8, N], F32, name="rn_out", addr_space="Shared")

            # ================= CONV1 =================
            with (
                tc.tile_pool(name="c1", bufs=1) as c1p,
                tc.tile_pool(name="xtra", bufs=1) as xtp,
            ):
                xt_sb = []
                for fc in range(8):
                    t = xtp.tile([128, 512], BF16, name=f"xt{fc}")
                    nc.sync.dma_start(t[:, :], xt_tgt[fc * 128:(fc + 1) * 128, :])
                    xt_sb.append(t)
                xe_sb = []
                for cc in range(4):
                    t = xtp.tile([128, N], BF16, name=f"xe{cc}")
                    nc.sync.dma_start(t[:, :], x_edge[cc * 128:(cc + 1) * 128, :])
                    xe_sb.append(t)

                def load_w(wt, n_k, nm):
                    ts_ = []
                    for fc in range(n_k):
                        t = wc.tile([128, 512], BF16, name=f"{nm}{fc}", tag="wc")
                        nc.sync.dma_start(t[:, :], wt[fc * 128:(fc + 1) * 128, :])
                        ts_.append(t)
                    return ts_

                # q projection (targets only): qT [512d, 512c]
                wq_sb = load_w(wq1, 8, "wq1_")
                qT = [c1p.tile([128, 512], BF16, name=f"qT{dc}") for dc in range(4)]
                for dc in range(4):
                    ps = pp.tile([128, 512], F32, name=f"psq{dc}", tag="mm")
                    for fc in range(8):
                        mmb(nc, ps[:, :], wq_sb[fc][:, dc * 128:(dc + 1) * 128],
                            xt_sb[fc][:, :], fc == 0, fc == 7)
                    nc.vector.tensor_scalar(qT[dc][:, :], ps[:, :],
                                            bq1c[:, dc:dc + 1], None, ALU.add)
                # k projection (all nodes): kT [512d, 1024r]
                wk_sb = load_w(wk1, 8, "wk1_")
                kT = [c1p.tile([128, N], BF16, name=f"kT{dc}") for dc in range(4)]
                for dc in range(4):
                    for rh in range(2):
                        ps = pp.tile([128, 512], F32, name=f"psk{dc}{rh}", tag="mm")
                        for fc in range(8):
                            mmb(nc, ps[:, :], wk_sb[fc][:, dc * 128:(dc + 1) * 128],
                                xT_sb[fc][:, rh * 512:(rh + 1) * 512], fc == 0, fc == 7)
                        nc.vector.tensor_scalar(kT[dc][:, rh * 512:(rh + 1) * 512], ps[:, :],
                                                bk1c[:, dc:dc + 1], None, ALU.add)
                # v natural [1024n, 512d] (bias folded into output bias)
                wv_sb = load_w(wv1, 8, "wv1_")
                v_bf = [c1p.tile([128, 512], BF16, name=f"v1_{nk}") for nk in range(8)]
                for nk in range(8):
                    ps = pp.tile([128, 512], F32, name=f"psv{nk}", tag="mm")
                    for fc in range(8):
                        mmb(nc, ps[:, :], xT_sb[fc][:, nk * 128:(nk + 1) * 128],
                            wv_sb[fc][:, :], fc == 0, fc == 7)
                    nc.vector.tensor_copy(v_bf[nk][:, :], ps[:, :])
                ws_sb = load_w(ws1, 8, "ws1_")

                # qe[c] = q_c . We  (col layout [128,4])
                qe_cols = smc.tile([128, 4], F32, name="qe_cols")
                for cc in range(4):
                    psq = pps.tile([128, 1], F32, name=f"psqe{cc}", tag="sm")
                    for dc in range(4):
                        mmf(nc, psq[:, :], qT[dc][:, cc * 128:(cc + 1) * 128],
                            we1cb[:, dc:dc + 1], dc == 0, dc == 3)
                    nc.scalar.activation(qe_cols[:, cc:cc + 1], psq[:, :], AF.Copy)

                # softmax + transpose, per 128-target chunk
                aT_bf = [c1p.tile([128, 512], BF16, name=f"aT1_{rc}") for rc in range(8)]
                t1_cols = smc.tile([128, 4], F32, name="t1_cols")
                for cc in range(4):
                    ps0 = pp.tile([128, 512], F32, name=f"psa{cc}", tag="mm")
                    ps1 = pp.tile([128, 512], F32, name=f"psb{cc}", tag="mm")
                    for dc in range(4):
                        mmf(nc, ps0[:, :], qT[dc][:, cc * 128:(cc + 1) * 128],
                            kT[dc][:, 0:512], dc == 0, dc == 3)
                    for dc in range(4):
                        mmf(nc, ps1[:, :], qT[dc][:, cc * 128:(cc + 1) * 128],
                            kT[dc][:, 512:1024], dc == 0, dc == 3)
                    xe32 = sm.tile([128, N], F32, name=f"xe32_{cc}", tag="xe32")
                    nc.scalar.activation(xe32[:, :], xe_sb[cc][:, :], AF.Copy)
                    ed = sm.tile([128, N], F32, name=f"ed{cc}", tag="ed")
                    nc.vector.tensor_scalar(ed[:, :], xe32[:, :],
                                            qe_cols[:, cc:cc + 1], None, ALU.mult)
                    al = sm.tile([128, N], F32, name=f"al{cc}", tag="al")
                    nc.vector.tensor_tensor(al[:, 0:512], ed[:, 0:512], ps0[:, :], ALU.add)
                    nc.vector.tensor_tensor(al[:, 512:1024], ed[:, 512:1024], ps1[:, :], ALU.add)
                    mcol = smc.tile([128, 1], F32, name=f"m{cc}")
                    nc.vector.reduce_max(mcol[:, :], al[:, :], axis=AX.X)
                    negms = smc.tile([128, 1], F32, name=f"nm{cc}")
                    nc.vector.tensor_scalar_mul(negms[:, :], mcol[:, :], -INV_S)
                    nc.scalar.activation(al[:, :], al[:, :], AF.Exp,
                                         bias=negms[:, :], scale=float(INV_S))
                    scol = smc.tile([128, 1], F32, name=f"s{cc}")
                    ucol = smc.tile([128, 1], F32, name=f"u{cc}")
                    nc.vector.reduce_sum(scol[:, :], al[:, :], axis=AX.X)
                    # u = alpha_exp * xe ; us = rowsum(u)  (fused)
                    nc.vector.tensor_tensor(ed[:, :], al[:, :], xe32[:, :], ALU.mult)
                    nc.vector.reduce_sum(ucol[:, :], ed[:, :], axis=AX.X)
                    rcol = smc.tile([128, 1], F32, name=f"r{cc}")
                    nc.vector.reciprocal(rcol[:, :], scol[:, :])
                    nc.vector.tensor_tensor(t1_cols[:, cc:cc + 1], ucol[:, :],
                                            rcol[:, :], ALU.mult)
                    nc.vector.tensor_scalar_mul(al[:, :], al[:, :], rcol[:, :])
                    for rc in range(8):
                        pst = ppt.tile([128, 128], F32, name=f"pt{cc}{rc}", tag="tr")
                        nc.tensor.transpose(pst[:, :], al[:, rc * 128:(rc + 1) * 128],
                                            ident[:, :])
                        nc.vector.tensor_copy(aT_bf[rc][:, cc * 128:(cc + 1) * 128],
                                              pst[:, :])
                # t row [1, 512]
                t1r = smc.tile([1, 512], BF16, name="t1r")
                pstr = pps.tile([1, 512], F32, name="pst1r", tag="sm")
                for cc in range(4):
                    nc.tensor.transpose(pstr[0:1, cc * 128:(cc + 1) * 128],
                                        t1_cols[:, cc:cc + 1], ident[:, :])
                nc.scalar.activation(t1r[0:1, :], pstr[0:1, :], AF.Copy)

                # output accumulation -> h1 block [512f, 512c]
                h1blk = [c1p.tile([128, 512], F32, name=f"h1b{dc}") for dc in range(4)]
                for dc in range(4):
                    ps = pp.tile([128, 512], F32, name=f"pso{dc}", tag="mm")
                    for rc in range(8):
                        mmb(nc, ps[:, :], v_bf[rc][:, dc * 128:(dc + 1) * 128],
                            aT_bf[rc][:, :], rc == 0, False)
                    mmf(nc, ps[:, :], we1rb[0:1, dc * 128:(dc + 1) * 128],
                        t1r[0:1, :], False, False)
                    for fc in range(8):
                        mmb(nc, ps[:, :], ws_sb[fc][:, dc * 128:(dc + 1) * 128],
                            xt_sb[fc][:, :], False, fc == 7)
                    nc.vector.tensor_scalar(h1blk[dc][:, :], ps[:, :],
                                            bvs1c[:, dc:dc + 1], None, ALU.add)

                # GraphNorm1 partial sums over my nodes
                S1c = smc.tile([128, 4], F32, name="S1c")
                S2c = smc.tile([128, 4], F32, name="S2c")
                for dc in range(4):
                    nc.vector.reduce_sum(S1c[:, dc:dc + 1], h1blk[dc][:, :], axis=AX.X)
                    sq = sm.tile([128, 512], F32, name=f"sq1_{dc}", tag="xe32")
                    nc.scalar.activation(sq[:, :], h1blk[dc][:, :], AF.Square)
                    nc.vector.reduce_sum(S2c[:, dc:dc + 1], sq[:, :], axis=AX.X)

                # ship block + stats
                for dc in range(4):
                    nc.sync.dma_start(ag_in[dc * 128:(dc + 1) * 128, :], h1blk[dc][:, :])
                nc.sync.dma_start(
                    ag_in[512, :].rearrange("(a p) -> p a", p=128), S1c[:, :])
                nc.sync.dma_start(
                    ag_in[513, :].rearrange("(a p) -> p a", p=128), S2c[:, :])

            nc.gpsimd.collective_compute(
                "AllGather", ALU.bypass,
                ins=[ag_in.opt()], outs=[ag_out.opt()],
                replica_groups=[list(range(N_CORES))],
            )

            # ---------- assemble full h1T (bf16) + GraphNorm1 ----------
            S1a = smc.tile([128, 16], F32, name="S1a")
            S2a = smc.tile([128, 16], F32, name="S2a")
            for j in range(8):
                hj, gj = j // 2, j % 2
                for dc in range(4):
                    stg = sm.tile([128, 512], F32, name=f"stg{j}_{dc}", tag="xe32")
                    nc.sync.dma_start(stg[:, :], ag_out[j, dc * 128:(dc + 1) * 128, :])
                    nc.vector.tensor_copy(
                        h1T[hj * 4 + dc][:, gj * 512:(gj + 1) * 512], stg[:, :])
            s1r = []
            s2r = []
            for j in range(8):
                a = smc.tile([128, 4], F32, name=f"s1r{j}")
                nc.sync.dma_start(a[:, :], ag_out[j, 512, :].rearrange("(a p) -> p a", p=128))
                s1r.append(a)
                b = smc.tile([128, 4], F32, name=f"s2r{j}")
                nc.sync.dma_start(b[:, :], ag_out[j, 513, :].rearrange("(a p) -> p a", p=128))
                s2r.append(b)
            for h in range(4):
                nc.vector.tensor_tensor(S1a[:, 4 * h:4 * h + 4], s1r[2 * h][:, :],
                                        s1r[2 * h + 1][:, :], ALU.add)
                nc.vector.tensor_tensor(S2a[:, 4 * h:4 * h + 4], s2r[2 * h][:, :],
                                        s2r[2 * h + 1][:, :], ALU.add)

            def gnorm_coeffs(S1t, S2t, gc, bc, mc, w, nm):
                mu = smc.tile([128, w], F32, name=f"mu{nm}")
                nc.vector.tensor_scalar_mul(mu[:, :], S1t[:, :], 1.0 / N)
                ex2 = smc.tile([128, w], F32, name=f"ex2{nm}")
                nc.vector.tensor_scalar_mul(ex2[:, :], S2t[:, :], 1.0 / N)
                msmu = smc.tile([128, w], F32, name=f"msmu{nm}")
                nc.vector.tensor_tensor(msmu[:, :], mc[:, :], mu[:, :], ALU.mult)
                tmp = smc.tile([128, w], F32, name=f"tmp{nm}")
                nc.vector.tensor_scalar_mul(tmp[:, :], mu[:, :], 2.0)
                nc.vector.tensor_tensor(tmp[:, :], tmp[:, :], msmu[:, :], ALU.subtract)
                nc.vector.tensor_tensor(tmp[:, :], msmu[:, :], tmp[:, :], ALU.mult)
                var = smc.tile([128, w], F32, name=f"var{nm}")
                nc.vector.tensor_tensor(var[:, :], ex2[:, :], tmp[:, :], ALU.subtract)
                nc.scalar.activation(var[:, :], var[:, :], AF.Sqrt, bias=eps_col[:, :])
                rstd = smc.tile([128, w], F32, name=f"rstd{nm}")
                nc.vector.reciprocal(rstd[:, :], var[:, :])
                scl = smc.tile([128, w], F32, name=f"scl{nm}")
                nc.vector.tensor_tensor(scl[:, :], gc[:, :], rstd[:, :], ALU.mult)
                sh = smc.tile([128, w], F32, name=f"sh{nm}")
                nc.vector.tensor_tensor(sh[:, :], scl[:, :], msmu[:, :], ALU.mult)
                nc.vector.tensor_tensor(sh[:, :], bc[:, :], sh[:, :], ALU.subtract)
                return scl, sh

            scl1, sh1 = gnorm_coeffs(S1a, S2a, gn1gc, gn1bc, gn1mc, 16, "g1")
            for f in range(16):
                nc.vector.tensor_scalar(h1T[f][:, :], h1T[f][:, :],
                                        scl1[:, f:f + 1], sh1[:, f:f + 1],
                                        ALU.mult, ALU.add)

            # ================= CONV2 =================
            with tc.tile_pool(name="c2", bufs=1) as c2p:
                def load_w2(wt, nm):
                    ts_ = []
                    for fc in range(16):
                        t = wc.tile([128, 512], BF16, name=f"{nm}{fc}", tag="wc")
                        nc.sync.dma_start(t[:, :], wt[fc * 128:(fc + 1) * 128, :])
                        ts_.append(t)
                    return ts_

                wq2_sb = load_w2(wq2, "wq2_")
                q2T = [c2p.tile([128, N], BF16, name=f"q2T{dc}") for dc in range(4)]
                for dc in range(4):
                    for ch in range(2):
                        ps = pp.tile([128, 512], F32, name=f"ps2q{dc}{ch}", tag="mm")
                        for fc in range(16):
                            mmb(nc, ps[:, :], wq2_sb[fc][:, dc * 128:(dc + 1) * 128],
                                h1T[fc][:, ch * 512:(ch + 1) * 512], fc == 0, fc == 15)
                        nc.vector.tensor_scalar(q2T[dc][:, ch * 512:(ch + 1) * 512], ps[:, :],
                                                bq2c[:, dc:dc + 1], None, ALU.add)
                wk2_sb = load_w2(wk2, "wk2_")
                k2T = [c2p.tile([128, N], BF16, name=f"k2T{dc}") for dc in range(4)]
                for dc in range(4):
                    for ch in range(2):
                        ps = pp.tile([128, 512], F32, name=f"ps2k{dc}{ch}", tag="mm")
                        for fc in range(16):
                            mmb(nc, ps[:, :], wk2_sb[fc][:, dc * 128:(dc + 1) * 128],
                                h1T[fc][:, ch * 512:(ch + 1) * 512], fc == 0, fc == 15)
                        nc.vector.tensor_scalar(k2T[dc][:, ch * 512:(ch + 1) * 512], ps[:, :],
                                                bk2c[:, dc:dc + 1], None, ALU.add)
                wv2_sb = load_w2(wv2, "wv2_")
                v2_bf = [c2p.tile([128, 512], BF16, name=f"v2_{nk}") for nk in range(8)]
                for nk in range(8):
                    ps = pp.tile([128, 512], F32, name=f"ps2v{nk}", tag="mm")
                    for fc in range(16):
                        mmb(nc, ps[:, :], h1T[fc][:, nk * 128:(nk + 1) * 128],
                            wv2_sb[fc][:, :], fc == 0, fc == 15)
                    nc.vector.tensor_copy(v2_bf[nk][:, :], ps[:, :])
                ws2_sb = load_w2(ws2, "ws2_")

                qe2 = smc.tile([128, 8], F32, name="qe2")
                for cc in range(8):
                    psq = pps.tile([128, 1], F32, name=f"ps2e{cc}", tag="sm")
                    for dc in range(4):
                        mmf(nc, psq[:, :], q2T[dc][:, cc * 128:(cc + 1) * 128],
                            we2cb[:, dc:dc + 1], dc == 0, dc == 3)
                    nc.scalar.activation(qe2[:, cc:cc + 1], psq[:, :], AF.Copy)

                aT2 = [c2p.tile([128, N], BF16, name=f"aT2_{rc}") for rc in range(8)]
                t2_cols = smc.tile([128, 8], F32, name="t2_cols")
                for cc in range(8):
                    ps0 = pp.tile([128, 512], F32, name=f"p2a{cc}", tag="mm")
                    ps1 = pp.tile([128, 512], F32, name=f"p2b{cc}", tag="mm")
                    for dc in range(4):
                        mmf(nc, ps0[:, :], q2T[dc][:, cc * 128:(cc + 1) * 128],
                            k2T[dc][:, 0:512], dc == 0, dc == 3)
                    for dc in range(4):
                        mmf(nc, ps1[:, :], q2T[dc][:, cc * 128:(cc + 1) * 128],
                            k2T[dc][:, 512:1024], dc == 0, dc == 3)
                    xe32 = sm.tile([128, N], F32, name=f"x2_{cc}", tag="xe32")
                    nc.scalar.activation(xe32[:, :], xT_sb[cc][:, :], AF.Copy)
                    ed = sm.tile([128, N], F32, name=f"ed2_{cc}", tag="ed")
                    nc.vector.tensor_scalar(ed[:, :], xe32[:, :],
                                            qe2[:, cc:cc + 1], None, ALU.mult)
                    al = sm.tile([128, N], F32, name=f"al2_{cc}", tag="al")
                    nc.vector.tensor_tensor(al[:, 0:512], ed[:, 0:512], ps0[:, :], ALU.add)
                    nc.vector.tensor_tensor(al[:, 512:1024], ed[:, 512:1024], ps1[:, :], ALU.add)
                    mcol = smc.tile([128, 1], F32, name=f"m2_{cc}")
                    nc.vector.reduce_max(mcol[:, :], al[:, :], axis=AX.X)
                    negms = smc.tile([128, 1], F32, name=f"nm2_{cc}")
                    nc.vector.tensor_scalar_mul(negms[:, :], mcol[:, :], -INV_S)
                    nc.scalar.activation(al[:, :], al[:, :], AF.Exp,
                                         bias=negms[:, :], scale=float(INV_S))
                    scol = smc.tile([128, 1], F32, name=f"s2_{cc}")
                    ucol = smc.tile([128, 1], F32, name=f"u2_{cc}")
                    nc.vector.reduce_sum(scol[:, :], al[:, :], axis=AX.X)
                    nc.vector.tensor_tensor(ed[:, :], al[:, :], xe32[:, :], ALU.mult)
                    nc.vector.reduce_sum(ucol[:, :], ed[:, :], axis=AX.X)
                    rcol = smc.tile([128, 1], F32, name=f"r2_{cc}")
                    nc.vector.reciprocal(rcol[:, :], scol[:, :])
                    nc.vector.tensor_tensor(t2_cols[:, cc:cc + 1], ucol[:, :],
                                            rcol[:, :], ALU.mult)
                    nc.vector.tensor_scalar_mul(al[:, :], al[:, :], rcol[:, :])
                    for rc in range(8):
                        pst = ppt.tile([128, 128], F32, name=f"p2t{cc}{rc}", tag="tr")
                        nc.tensor.transpose(pst[:, :], al[:, rc * 128:(rc + 1) * 128],
                                            ident[:, :])
                        nc.vector.tensor_copy(aT2[rc][:, cc * 128:(cc + 1) * 128],
                                              pst[:, :])
                t2r = smc.tile([1, N], BF16, name="t2r")
                for nh in range(2):
                    pstr = pps.tile([1, 512], F32, name=f"pst2r{nh}", tag="sm")
                    for cc in range(4):
                        nc.tensor.transpose(pstr[0:1, cc * 128:(cc + 1) * 128],
                                            t2_cols[:, nh * 4 + cc:nh * 4 + cc + 1],
                                            ident[:, :])
                    nc.scalar.activation(t2r[0:1, nh * 512:(nh + 1) * 512],
                                         pstr[0:1, :], AF.Copy)

                h2T = [c2p.tile([128, N], F32, name=f"h2T{dc}") for dc in range(4)]
                for dc in range(4):
                    for ch in range(2):
                        ps = pp.tile([128, 512], F32, name=f"ps2o{dc}{ch}", tag="mm")
                        for rc in range(8):
                            mmb(nc, ps[:, :], v2_bf[rc][:, dc * 128:(dc + 1) * 128],
                                aT2[rc][:, ch * 512:(ch + 1) * 512], rc == 0, False)
                        mmf(nc, ps[:, :], we2rb[0:1, dc * 128:(dc + 1) * 128],
                            t2r[0:1, ch * 512:(ch + 1) * 512], False, False)
                        for fc in range(16):
                            mmb(nc, ps[:, :], ws2_sb[fc][:, dc * 128:(dc + 1) * 128],
                                h1T[fc][:, ch * 512:(ch + 1) * 512], False, fc == 15)
                        nc.vector.tensor_scalar(h2T[dc][:, ch * 512:(ch + 1) * 512], ps[:, :],
                                                bvs2c[:, dc:dc + 1], None, ALU.add)

                # GraphNorm2 (local)
                T1 = smc.tile([128, 4], F32, name="T1")
                T2 = smc.tile([128, 4], F32, name="T2")
                for dc in range(4):
                    nc.vector.reduce_sum(T1[:, dc:dc + 1], h2T[dc][:, :], axis=AX.X)
                    sq = sm.tile([128, N], F32, name=f"sq2_{dc}", tag="ed")
                    nc.scalar.activation(sq[:, :], h2T[dc][:, :], AF.Square)
                    nc.vector.reduce_sum(T2[:, dc:dc + 1], sq[:, :], axis=AX.X)
                scl2, sh2 = gnorm_coeffs(T1, T2, gn2gc, gn2bc, gn2mc, 4, "g2")
                for dc in range(4):
                    nc.vector.tensor_scalar(h2T[dc][:, :], h2T[dc][:, :],
                                            scl2[:, dc:dc + 1], sh2[:, dc:dc + 1],
                                            ALU.mult, ALU.add)

                # row-norm partial sumsq (over my 512 features) via ones-matmul
                rn_row = smc.tile([1, N], F32, name="rn_row")
                for nh in range(2):
                    psr = pps.tile([1, 512], F32, name=f"psrn{nh}", tag="sm")
                    for dc in range(4):
                        sqh = sm.tile([128, 512], BF16, name=f"sqh{nh}{dc}", tag="sqh")
                        nc.scalar.activation(sqh[:, :],
                                             h2T[dc][:, nh * 512:(nh + 1) * 512],
                                             AF.Square)
                        mmf(nc, psr[0:1, :], ones_col[:, :], sqh[:, :],
                            dc == 0, dc == 3)
                    nc.scalar.activation(rn_row[0:1, nh * 512:(nh + 1) * 512],
                                         psr[0:1, :], AF.Copy)
                nc.sync.dma_start(rn_in[0:1, :], rn_row[0:1, :])

                nc.gpsimd.collective_compute(
                    "AllGather", ALU.bypass,
                    ins=[rn_in.opt()], outs=[rn_out.opt()],
                    replica_groups=[list(range(N_CORES))],
                )

                tot = smc.tile([128, 8], F32, name="tot")
                prev = None
                for j in range(8):
                    rsj = smc.tile([128, 8], F32, name=f"rs{j}")
                    nc.sync.dma_start(rsj[:, :],
                                      rn_out[j, :].rearrange("(a p) -> p a", p=128))
                    if j == 0:
                        prev = rsj
                    elif j == 1:
                        nc.vector.tensor_tensor(tot[:, :], prev[:, :], rsj[:, :], ALU.add)
                    else:
                        nc.vector.tensor_tensor(tot[:, :], tot[:, :], rsj[:, :], ALU.add)
                nc.scalar.activation(tot[:, :], tot[:, :], AF.Sqrt)
                inv = smc.tile([128, 8], F32, name="inv")
                nc.vector.reciprocal(inv[:, :], tot[:, :])

                # transpose to natural [node, feat] layout, scale, store
                for dc in range(4):
                    for nk in range(8):
                        pst = ppt.tile([128, 128], F32, name=f"pf{dc}{nk}", tag="tr")
                        nc.tensor.transpose(pst[:, :], h2T[dc][:, nk * 128:(nk + 1) * 128],
                                            ident[:, :])
                        ob = sm.tile([128, 128], F32, name=f"ob{dc}{nk}", tag="ob")
                        nc.vector.tensor_scalar_mul(ob[:, :], pst[:, :],
                                                    inv[:, nk:nk + 1])
                        nc.sync.dma_start(
                            out[nk * 128:(nk + 1) * 128, dc * 128:(dc + 1) * 128],
                            ob[:, :])
    return nc


_NC_CACHE = None


def _get_nc():
    global _NC_CACHE
    if _NC_CACHE is None:
        nc = build_nc()
        # local walrus only accepts one sync-wait per CTRL-class instruction
        for f in nc.m.functions:
            for bb in f.blocks:
                changed = False
                new_list = []
                for ins in bb.instructions:
                    si = ins.sync_info
                    if si is not None and len(si.on_wait) > 1:
                        waits = list(si.on_wait)
                        for i, w in enumerate(waits[:-1]):
                            nop = mybir.InstNoOp(
                                name=f"{ins.name}_presplit{i}", engine=ins.engine)
                            nop.sync_info = mybir.SyncInfo(on_wait=[w], on_update=[])
                            new_list.append(nop)
                        ins.sync_info = mybir.SyncInfo(
                            on_wait=[waits[-1]], on_update=list(si.on_update))
                        changed = True
                    new_list.append(ins)
                if changed:
                    bb.instructions = new_list
        _NC_CACHE = nc
    return _NC_CACHE


def kernel(**inputs):
    x = np.asarray(inputs["x"], np.float32)
    bf = ml_dtypes.bfloat16

    def c(a, dt=np.float32):
        return np.ascontiguousarray(a).astype(dt)

    xT = np.ascontiguousarray(x.T)
    in_maps = []
    for i in range(N_CORES):
        h, g = i // 2, i % 2
        s1, s2i = slice(512 * h, 512 * (h + 1)), slice(512 * i, 512 * (i + 1))
        m = {
            "xT": c(xT, bf),
            "xt_tgt": c(xT[:, 512 * g:512 * (g + 1)], bf),
            "x_edge": c(xT[512 * g:512 * (g + 1), :], bf),
            "wq1": c(inputs["q1_w"][:, s1], bf),
            "wk1": c(inputs["k1_w"][:, s1], bf),
            "wv1": c(inputs["v1_w"][:, s1], bf),
            "ws1": c(inputs["s1_w"][:, s1], bf),
            "bq1": c(inputs["q1_b"][s1]),
            "bk1": c(inputs["k1_b"][s1]),
            "bvs1": c(np.asarray(inputs["v1_b"][s1], np.float32)
                      + np.asarray(inputs["s1_b"][s1], np.float32)),
            "we1": c(np.asarray(inputs["e1_w"], np.float32).reshape(4, 512)[h]),
            "gn1g": c(inputs["gn1_gamma"]),
            "gn1b": c(inputs["gn1_beta"]),
            "gn1m": c(inputs["gn1_ms"]),
            "wq2": c(inputs["q2_w"][:, s2i], bf),
            "wk2": c(inputs["k2_w"][:, s2i], bf),
            "wv2": c(inputs["v2_w"][:, s2i], bf),
            "ws2": c(inputs["s2_w"][:, s2i], bf),
            "bq2": c(inputs["q2_b"][s2i]),
            "bk2": c(inputs["k2_b"][s2i]),
            "bvs2": c(np.asarray(inputs["v2_b"][s2i], np.float32)
                      + np.asarray(inputs["s2_b"][s2i], np.float32)),
            "we2": c(np.asarray(inputs["e2_w"], np.float32).reshape(8, 512)[i]),
            "gn2g": c(inputs["gn2_gamma"][s2i]),
            "gn2b": c(inputs["gn2_beta"][s2i]),
            "gn2m": c(inputs["gn2_ms"][s2i]),
        }
        in_maps.append(m)

    res = _run_cached(in_maps)
    full = np.empty((N, 2 * HR), np.float32)
    for i in range(N_CORES):
        full[:, 512 * i:512 * (i + 1)] = res[i]["out"]
    return full


_RUNNER = None


def _get_runner():
    """Build the sharded jitted executable once per process."""
    global _RUNNER
    if _RUNNER is not None:
        return _RUNNER
    import jax
    from jax.sharding import Mesh, PartitionSpec, NamedSharding
    from jax.experimental.shard_map import shard_map
    from concourse import bass2jax
    from concourse.bass2jax import _bass_exec_p, install_neuronx_cc_hook

    nc = _get_nc()
    install_neuronx_cc_hook()
    partition_name = nc.partition_id_tensor.name if nc.partition_id_tensor else None
    in_names, out_names, out_avals, zero_outs = [], [], [], []
    for alloc in nc.m.functions[0].allocations:
        if not isinstance(alloc, mybir.MemoryLocationSet):
            continue
        name = alloc.memorylocations[0].name
        if alloc.kind == "ExternalInput":
            if name != partition_name:
                in_names.append(name)
        elif alloc.kind == "ExternalOutput":
            out_names.append(name)
            out_avals.append(jax.core.ShapedArray(
                tuple(alloc.tensor_shape), mybir.dt.np(alloc.dtype)))
            zero_outs.append(np.zeros(tuple(alloc.tensor_shape),
                                      mybir.dt.np(alloc.dtype)))
    n_params, n_outs = len(in_names), len(out_avals)
    all_names = in_names + out_names + ([partition_name] if partition_name else [])
    donate = tuple(range(n_params, n_params + n_outs))

    def _body(*args):
        operands = list(args)
        if partition_name is not None:
            operands.append(bass2jax.partition_id_tensor())
        return tuple(_bass_exec_p.bind(
            *operands, out_avals=tuple(out_avals), in_names=tuple(all_names),
            out_names=tuple(out_names), lowering_input_output_aliases=(),
            sim_require_finite=True, sim_require_nnan=True, nc=nc))

    devices = jax.devices()[:N_CORES]
    mesh = Mesh(np.asarray(devices), ("core",))
    sharded = jax.jit(
        shard_map(_body, mesh=mesh,
                  in_specs=(PartitionSpec("core"),) * (n_params + n_outs),
                  out_specs=(PartitionSpec("core"),) * n_outs,
                  check_rep=False),
        donate_argnums=donate, keep_unused=True)
    sh = NamedSharding(mesh, PartitionSpec("core"))
    _RUNNER = (sharded, sh, in_names, out_names, out_avals, zero_outs, jax)
    return _RUNNER


def _run_cached(in_maps):
    sharded, sh, in_names, out_names, out_avals, zero_outs, jax = _get_runner()
    concat_in = [np.concatenate([np.asarray(in_maps[c][nm])
                                 for c in range(N_CORES)], axis=0)
                 for nm in in_names]
    dev_in = [jax.device_put(a, sh) for a in concat_in]
    zs = [jax.device_put(np.zeros((N_CORES * z.shape[0], *z.shape[1:]), z.dtype), sh)
          for z in zero_outs]
    outs = sharded(*dev_in, *zs)
    outs = [np.asarray(o).reshape(N_CORES, *out_avals[i].shape)
            for i, o in enumerate(outs)]
    return [{nm: outs[i][c] for i, nm in enumerate(out_names)}
            for c in range(N_CORES)]

